# revision 21
# baseline (speedup 1.0000x reference)
"""Trainium2 Bass kernel for nn_MetaLearningWithMemory (v3).

Data-parallel over the query batch across 8 cores.  Key structure:
  - x is transposed + fp16-cast on the host (no on-device transposes, half
    the HBM traffic for x); wenc/x loads are split into quarter-tiles and
    interleaved so the first encoder chain starts ~3us into the kernel.
  - attention runs in the transposed [m, b] layout: scores^T via fp8
    matmuls, exp without accum_out, per-head softmax sums via a leading
    ones-column in the fp8 DoubleRow value matmul, and normalization
    deferred past tiny per-head classifier matmuls (b_cls folded in via the
    sum-row of the augmented classifier weights; b_enc folded into bq/bcls
    on the host).  The per-head z matmuls and sum-row transposes are
    emitted immediately after each head's value matmul; the final per-bt
    assembly is one broadcast multiply + strided reduce on DVE.
  - qf and the value stage use fp8 DoubleRow (2x PE rate).
  - emission is software-pipelined: attention of chunk N is interleaved
    into the encoder matmul chains of chunk N+1 (value matmuls deferred one
    slot behind their scores/exp), the scan's serial Jacobi groups are
    interleaved the same way, and the last chunk's attention rides inside
    its own macro so the epilogue is only the tiny z-assembly.
"""

from contextlib import ExitStack

import numpy as np

import concourse.bass as bass
import concourse.mybir as mybir
import concourse.tile as tile
from concourse import bacc
from concourse.bass_utils import run_bass_kernel_spmd

D_IN = 2048
F = 512
M = 256
NS = 512
H = 8
DH = 64
NW = 5
B = 16384
NCORES = 8
BC = B // NCORES          # 2048 batch rows per core
NB = 512                  # batch chunk
NCH = BC // NB            # 4 chunks
NITER = 2                 # Jacobi softmax passes (converges at 2)
INV_SQRT_F = float(F) ** -0.5
INV_SQRT_DH = float(DH) ** -0.5

f16 = mybir.dt.float16
f32 = mybir.dt.float32
f8 = mybir.dt.float8e4
FT = mybir.ActivationFunctionType
DR = mybir.MatmulPerfMode.DoubleRow
E4NP = mybir.dt.np(f8)


def _f16(a):
    return np.asarray(a, dtype=np.float32).astype(np.float16)


def _f8(a):
    return np.asarray(a, dtype=np.float32).astype(E4NP)


def build(stage="full", repeat=1, opts=None):
    opts = dict(opts or {})
    O = lambda k, d: opts.get(k, d)
    niter = O("niter", NITER)

    nc = bacc.Bacc("TRN2", target_bir_lowering=False)

    # ---- per-core external inputs (host-prepped layouts) ----
    xs = nc.dram_tensor("xs", [128, NCH, 16, NB], f16, kind="ExternalInput")
    wenc = nc.dram_tensor("wenc", [128, 16, F], f16, kind="ExternalInput")
    sxt = nc.dram_tensor("sxt", [128, 16, NS], f16, kind="ExternalInput")
    wq8 = nc.dram_tensor("wq8", [128, 4, F], f8, kind="ExternalInput")
    wclst = nc.dram_tensor("wclst", [128, 4, NW], f16, kind="ExternalInput")
    w2aug = nc.dram_tensor("w2aug", [65, H, NW], f16, kind="ExternalInput")
    mkt0 = nc.dram_tensor("mkt0", [128, 4, M], f32, kind="ExternalInput")
    mkt0b = nc.dram_tensor("mkt0b", [128, 4, M], f16, kind="ExternalInput")
    mvals = nc.dram_tensor("mvals", [128, 2, F], f16, kind="ExternalInput")
    valsb = nc.dram_tensor("valsb", [128, 4, NW], f16, kind="ExternalInput")
    benc = nc.dram_tensor("benc", [128, 4], f32, kind="ExternalInput")
    bq = nc.dram_tensor("bq", [128, 4], f32, kind="ExternalInput")
    y = nc.dram_tensor("y", [BC, NW], f32, kind="ExternalOutput")

    eye128 = nc.inline_tensor(np.eye(128, dtype=np.float16), name="eye128")
    # mask_su[s, t] = inv_sqrt_f if s < t else 0 (strict upper in [s, t])
    mask_np = np.triu(np.full((128, 128), INV_SQRT_F, np.float32), 1).astype(np.float16)
    mask_su = nc.inline_tensor(mask_np, name="mask_su")

    NMAC = NCH * repeat

    with tile.TileContext(nc) as tc:
        with ExitStack() as ctx:
            ep = ctx.enter_context
            const = ep(tc.tile_pool(name="const", bufs=1))
            persist = ep(tc.tile_pool(name="persist", bufs=1))
            xt_pool = ep(tc.tile_pool(name="xt", bufs=O("xt", 16)))
            ft_pool = ep(tc.tile_pool(name="ft16", bufs=O("ft16", 9)))
            f8_pool = ep(tc.tile_pool(name="ft8", bufs=O("ft8", 2)))
            qf_pool = ep(tc.tile_pool(name="qf8", bufs=O("qf8", 2)))
            u8_pool = ep(tc.tile_pool(name="u8", bufs=O("u8", 6)))
            mo_pool = ep(tc.tile_pool(name="mo", bufs=O("mo", 6)))
            r_pool = ep(tc.tile_pool(name="r", bufs=4))
            zs_pool = ep(tc.tile_pool(name="zs", bufs=4))
            tq_pool = ep(tc.tile_pool(name="tq", bufs=4))
            y_pool = ep(tc.tile_pool(name="ysb", bufs=2))
            w_pool = ep(tc.tile_pool(name="w", bufs=2))
            usb_pool = ep(tc.tile_pool(name="usb", bufs=4))
            rs_pool = ep(tc.tile_pool(name="rs", bufs=4))
            psA = ep(tc.tile_pool(name="psA", bufs=O("psA", 2), space="PSUM"))
            psS = ep(tc.tile_pool(name="psS", bufs=O("psS", 3), space="PSUM"))
            psV = ep(tc.tile_pool(name="psV", bufs=O("psV", 1), space="PSUM"))
            psZ = ep(tc.tile_pool(name="psZ", bufs=O("psZ", 1), space="PSUM"))

            # ============ constant + x loads, bandwidth-ordered ============
            # wenc and xt0 quarter-tiles interleaved so the chunk-0 encoder
            # can start after ~1/4 of each has landed; sxt next (scan); the
            # small constants ride behind.
            def load_xt_quarters(ch):
                parts = []
                for jp in range(4):
                    xt = xt_pool.tile([128, 4, NB], f16, tag="xt")
                    nc.gpsimd.dma_start(xt[:], xs[:, ch, 4 * jp:4 * jp + 4])
                    parts.append(xt)
                return parts

            wenc_t = []
            xt0_parts = []
            sxt_t = []
            for jp in range(4):
                wt = const.tile([128, 4, F], f16, tag="wenct", bufs=4,
                                name=f"wenc{jp}")
                nc.sync.dma_start(wt[:], wenc[:, 4 * jp:4 * jp + 4])
                wenc_t.append(wt)
                xt = xt_pool.tile([128, 4, NB], f16, tag="xt")
                nc.gpsimd.dma_start(xt[:], xs[:, 0, 4 * jp:4 * jp + 4])
                xt0_parts.append(xt)
                st = const.tile([128, 4, NS], f16, tag="sxtt", bufs=4,
                                name=f"sxt{jp}")
                nc.sync.dma_start(st[:], sxt[:, 4 * jp:4 * jp + 4])
                sxt_t.append(st)
            xts = [xt0_parts]
            if NMAC > 1:
                xts.append(load_xt_quarters(1))
            wq8_sb = const.tile([128, 4, F], f8)
            nc.sync.dma_start(wq8_sb[:], wq8[:])
            wclst_sb = const.tile([128, 4, NW], f16)
            nc.sync.dma_start(wclst_sb[:], wclst[:])
            w2aug_sb = const.tile([65, H, NW], f16)
            nc.sync.dma_start(w2aug_sb[:], w2aug[:])
            benc_sb = const.tile([128, 4], f32)
            nc.sync.dma_start(benc_sb[:], benc[:])
            bq_sb = const.tile([128, 4], f32)
            nc.sync.dma_start(bq_sb[:], bq[:])
            eye128_sb = const.tile([128, 128], f16)
            nc.sync.dma_start(eye128_sb[:], eye128[:])
            mask_sb = const.tile([128, 128], f16)
            nc.sync.dma_start(mask_sb[:], mask_su[:])
            mkt0b_sb = const.tile([128, 4, M], f16)
            nc.sync.dma_start(mkt0b_sb[:], mkt0b[:])
            for ch in range(2, min(NCH, NMAC)):
                xts.append(load_xt_quarters(ch))
            mkt0_sb = const.tile([128, 4, M], f32)
            nc.sync.dma_start(mkt0_sb[:], mkt0[:])
            mv_sb = const.tile([128, 2, F], f16)
            nc.sync.dma_start(mv_sb[:], mvals[:])
            vals_sb = const.tile([128, 4, NW], f16)
            nc.sync.dma_start(vals_sb[:], valsb[:])

            # persistent scan results
            st_bf = persist.tile([128, 4, NS], f16, name="st_bf")
            s_sb = persist.tile([128, 4, F], f16, name="s_sb")
            g_sb = persist.tile([128, 4, NS], f16, name="g_sb")
            base_sb = persist.tile([128, 4, M], f32, name="base_sb")
            mkt8 = persist.tile([128, 4, M], f8, name="mkt8")
            mv_nat = persist.tile([128, 2, F], f16, name="mv_nat")
            mv8aug = persist.tile([128, 2, H, 80], f8, name="mv8aug")

            # ================= scan part A (batch-shaped) =================
            def scan_a():
                # S^T [f, t] = W_enc^T Sx^T; + b_enc
                for ft in range(4):
                    ps = psA.tile([128, NB], f32, tag="psA")
                    for j in range(16):
                        nc.tensor.matmul(
                            ps[:], wenc_t[j // 4][:, j % 4, ft * 128:(ft + 1) * 128],
                            sxt_t[j // 4][:, j % 4, :],
                            start=(j == 0), stop=(j == 15),
                        )
                    nc.scalar.activation(st_bf[:, ft, :], ps[:], FT.Identity,
                                         bias=benc_sb[:, ft:ft + 1])
                # S natural [t, f] via PE transpose of S^T
                for tt in range(4):
                    pt = psS.tile([128, NB], f16, tag="psS")
                    for ft in range(4):
                        nc.tensor.transpose(
                            pt[:, ft * 128:(ft + 1) * 128],
                            st_bf[:, ft, tt * 128:(tt + 1) * 128], eye128_sb[:])
                    nc.vector.tensor_copy(s_sb[:, tt, :], pt[:])
                # G[s, t] = (S S^T)/sqrt(F); diag blocks masked strict-upper
                for ks in range(4):
                    ps = psA.tile([128, NB], f32, tag="psA")
                    for kf in range(4):
                        nc.tensor.matmul(
                            ps[:], st_bf[:, kf, ks * 128:(ks + 1) * 128],
                            st_bf[:, kf, :], start=(kf == 0), stop=(kf == 3),
                        )
                    for tt in range(4):
                        dst = g_sb[:, ks, tt * 128:(tt + 1) * 128]
                        src = ps[:, tt * 128:(tt + 1) * 128]
                        if tt == ks:
                            nc.vector.tensor_mul(dst, src, mask_sb[:])
                        elif tt > ks:
                            nc.vector.tensor_scalar_mul(dst, src, INV_SQRT_F)
                # base[t, m] = S @ mem_keys^T / sqrt(F)
                for tt in range(4):
                    pl = psS.tile([128, M], f32, tag="psS")
                    for kf in range(4):
                        nc.tensor.matmul(
                            pl[:], st_bf[:, kf, tt * 128:(tt + 1) * 128],
                            mkt0b_sb[:, kf, :], start=(kf == 0), stop=(kf == 3),
                        )
                    nc.scalar.copy(base_sb[:, tt, :], pl[:])

            # ============ scan part B: serial Jacobi groups ============
            # returned as closures; interleaved into macro 1's encoder chains
            scan_groups = []
            w_tiles = [None]

            def jac0(tt):
                def g():
                    if tt == 0:
                        w_tiles[0] = w_pool.tile([128, 4, M], f16, tag="wt",
                                                 name="w_it0")
                    w_cur = w_tiles[0]
                    u = usb_pool.tile([128, M], f32, tag="usb")
                    ssum = rs_pool.tile([128, 1], f32, tag="scol")
                    nc.scalar.activation(u[:], base_sb[:, tt, :], FT.Exp,
                                         accum_out=ssum[:])
                    rcol = rs_pool.tile([128, 1], f32, tag="scol")
                    nc.vector.reciprocal(rcol[:], ssum[:])
                    nc.vector.tensor_scalar_mul(w_cur[:, tt, :], u[:], rcol[:])
                return g

            def jac(it, tt):
                def g():
                    w_cur = w_tiles[0]
                    if tt == 0:
                        w_tiles.append(w_pool.tile([128, 4, M], f16, tag="wt",
                                                   name=f"w_it{it}"))
                    w_new = w_tiles[-1]
                    pl = psS.tile([128, M], f32, tag="psS")
                    for ks in range(tt + 1):
                        nc.tensor.matmul(
                            pl[:], g_sb[:, ks, tt * 128:(tt + 1) * 128],
                            w_cur[:, ks, :], start=(ks == 0), stop=(ks == tt),
                        )
                    nc.vector.tensor_add(pl[:], pl[:], base_sb[:, tt, :])
                    u = usb_pool.tile([128, M], f32, tag="usb")
                    ssum = rs_pool.tile([128, 1], f32, tag="scol")
                    nc.scalar.activation(u[:], pl[:], FT.Exp, accum_out=ssum[:])
                    rcol = rs_pool.tile([128, 1], f32, tag="scol")
                    nc.vector.reciprocal(rcol[:], ssum[:])
                    nc.vector.tensor_scalar_mul(w_new[:, tt, :], u[:], rcol[:])
                    if tt == 3:
                        w_tiles[0] = w_new
                return g

            def mk_group(ft):
                def g():
                    w_cur = w_tiles[0]
                    pl = psS.tile([128, M], f32, tag="psS")
                    for tt in range(4):
                        nc.tensor.matmul(
                            pl[:], s_sb[:, tt, ft * 128:(ft + 1) * 128],
                            w_cur[:, tt, :], start=(tt == 0), stop=(tt == 3),
                        )
                    nc.vector.tensor_add(mkt8[:, ft, :], pl[:],
                                         mkt0_sb[:, ft, :])
                return g

            def mv_group(mt):
                def g():
                    w_cur = w_tiles[0]
                    nc.vector.tensor_copy(mv_nat[:, mt, NW:], mv_sb[:, mt, NW:])
                    pl = psS.tile([128, M], f32, tag="psS")
                    for tt in range(4):
                        nc.tensor.matmul(
                            pl[:, 0:NW], w_cur[:, tt, mt * 128:(mt + 1) * 128],
                            vals_sb[:, tt, :], start=(tt == 0), stop=(tt == 3),
                        )
                    nc.vector.tensor_add(mv_nat[:, mt, 0:NW], pl[:, 0:NW],
                                         mv_sb[:, mt, 0:NW])
                return g

            def mv8_group():
                def g():
                    # aug layout per head: col 0 = ones, cols 1..65 = values
                    nc.vector.memset(mv8aug[:, :, :, 0:1], 1.0)
                    nc.vector.tensor_copy(
                        mv8aug[:, :, :, 1:DH + 1],
                        mv_nat[:].rearrange("p mt (h d) -> p mt h d", d=DH))
                return g

            for tt in range(4):
                scan_groups.append(jac0(tt))
            for it in range(1, niter):
                for tt in range(4):
                    scan_groups.append(jac(it, tt))
            for ft in range(4):
                scan_groups.append(mk_group(ft))
            for mt in range(2):
                scan_groups.append(mv_group(mt))
            scan_groups.append(mv8_group())

            # ================= main pipeline stages =================
            def feat_chain(xt_parts, ft, feat16, feat8):
                ps = psA.tile([128, NB], f32, tag="psA")
                for j in range(16):
                    nc.tensor.matmul(
                        ps[:], wenc_t[j // 4][:, j % 4, ft * 128:(ft + 1) * 128],
                        xt_parts[j // 4][:, j % 4, :],
                        start=(j == 0), stop=(j == 15),
                    )
                f16t = ft_pool.tile([128, NB], f16, tag="ft16")
                nc.scalar.copy(f16t[:], ps[:])
                feat16.append(f16t)
                nc.vector.tensor_copy(feat8[:, ft, :], ps[:])

            def qf_chain(feat8, ft, qf8):
                ps = psA.tile([128, NB], f32, tag="psA")
                for u2 in range(2):
                    nc.tensor.matmul(
                        ps[:], wq8_sb[:, 2 * u2:2 * u2 + 2,
                                      ft * 128:(ft + 1) * 128],
                        feat8[:, 2 * u2:2 * u2 + 2, :],
                        start=(u2 == 0), stop=(u2 == 1), perf_mode=DR,
                    )
                nc.vector.tensor_scalar_add(qf8[:, ft, :], ps[:],
                                            bq_sb[:, ft:ft + 1])

            def attn_setup(feat16):
                # logits1 accumulation + per-bt z column groups, one bank
                pz4 = psZ.tile([128, 4, 48], f32, tag="pz")
                pr4 = psV.tile([128, 4, H, 2], f16, tag="pr", bufs=1)
                for bt in range(4):
                    bsl = slice(bt * 128, (bt + 1) * 128)
                    for k in range(4):
                        nc.tensor.matmul(pz4[:, bt, 40:45],
                                         feat16[k][:, bsl], wclst_sb[:, k, :],
                                         start=(k == 0), stop=(k == 3))
                return dict(pz4=pz4, pr4=pr4, zq=[])

            def attn_scores(qf8, h, us):
                kf, p0 = h // 2, 64 * (h % 2)
                u8 = u8_pool.tile([128, 2, NB], f8, tag="u8")
                for mj in range(2):
                    ps = psS.tile([128, NB], f32, tag="psS")
                    nc.tensor.matmul(
                        ps[:],
                        mkt8[p0:p0 + 64, kf, mj * 128:(mj + 1) * 128],
                        qf8[p0:p0 + 64, kf, :], start=True, stop=True,
                    )
                    nc.scalar.activation(u8[:, mj, :], ps[:], FT.Exp,
                                         scale=INV_SQRT_DH)
                us.append((h, u8))

            def attn_value(h, u8, actx):
                pv = psV.tile([65, NB], f32, tag="psV")
                nc.tensor.matmul(pv[:], mv8aug[:, :, h, 0:DH + 1], u8[:],
                                 start=True, stop=True, perf_mode=DR)
                mo = mo_pool.tile([65, NB], f16, tag="mo")
                if h % 2 == 0:
                    nc.scalar.copy(mo[:], pv[:])
                else:
                    nc.vector.tensor_copy(mo[:], pv[:])
                actx["zq"].append((h, mo))

            def emit_z(h, mo, actx):
                pz4, pr4 = actx["pz4"], actx["pr4"]
                for bt in range(4):
                    bsl = slice(bt * 128, (bt + 1) * 128)
                    nc.tensor.transpose(pr4[:, bt, h, 0:1], mo[0:1, bsl],
                                        eye128_sb[0:1, 0:1])
                    nc.tensor.matmul(pz4[:, bt, 5 * h:5 * h + 5], mo[:, bsl],
                                     w2aug_sb[:, h, :], start=True, stop=True)

            def drain_z(actx, keep=0):
                zq = actx["zq"]
                while len(zq) > keep:
                    h, mo = zq.pop(0)
                    emit_z(h, mo, actx)

            def drain_values(us, actx, keep=0):
                while len(us) > keep:
                    h, u8 = us.pop(0)
                    attn_value(h, u8, actx)
                    drain_z(actx, keep=1)

            def attn_finish(actx, pch):
                # stage-major emission: consecutive DVE ops are independent
                # so they pipeline instead of paying the dep-chain latency
                pz4, pr4 = actx["pz4"], actx["pr4"]
                ysb = y_pool.tile([128, NCH, NW], f32, tag="ysb")
                rs, zss, tqs = [], [], []
                for bt in range(4):
                    r = r_pool.tile([128, 1, H], f32, tag="r")
                    nc.vector.reciprocal(
                        r[:],
                        pr4[:, bt, :, 0:1].rearrange("p h one -> p one h"))
                    rs.append(r)
                for bt in range(4):
                    zs = zs_pool.tile([128, NW, H], f32, tag="zs")
                    zv, rv = bass.broadcast_tensor_aps(
                        pz4[:, bt, 0:40].rearrange("p (h c) -> p c h", c=NW),
                        rs[bt][:])
                    nc.vector.tensor_mul(zs[:], zv, rv)
                    zss.append(zs)
                for bt in range(4):
                    tq = tq_pool.tile([128, NW], f32, tag="tq")
                    nc.vector.tensor_reduce(tq[:], zss[bt][:],
                                            axis=mybir.AxisListType.X,
                                            op=mybir.AluOpType.add)
                    tqs.append(tq)
                for bt in range(4):
                    nc.vector.tensor_add(ysb[:, bt, :], tqs[bt][:],
                                         pz4[:, bt, 40:45])
                nc.sync.dma_start(
                    y[pch * NB:(pch + 1) * NB, :].rearrange(
                        "(bt p) c -> p bt c", p=128),
                    ysb[:])

            # ================= software-pipelined macro loop =================
            # macro i: encoder+qf for chunk i; attention+z+y for chunk i-1
            # interleaved into the chains.  Macro 1 runs chunk 0's attention
            # in its qf phase (the feat phase carries the scan's serial
            # groups); the last encoder macro additionally runs its own
            # chunk's attention in its qf phase so the epilogue is only the
            # z-assembly.
            state = {}
            pending = list(scan_groups)

            for i in range(NMAC + 1):
                enc = i < NMAC
                last_enc = i == NMAC - 1
                ch = i % NCH
                pch = (i - 1) % NCH
                pstate = state
                state = {}
                us = []
                actx = None
                if i >= 2 and "us" not in pstate:
                    actx = attn_setup(pstate["feat16"])
                if enc:
                    xt_parts = xts[ch]
                    feat16 = []
                    feat8 = f8_pool.tile([128, 4, NB], f8, tag="ft8")
                    qf8 = qf_pool.tile([128, 4, NB], f8, tag="qf8")
                    state = dict(feat16=feat16, qf8=qf8)
                    for ft in range(4):
                        feat_chain(xt_parts, ft, feat16, feat8)
                        if i == 1:
                            # interleave serial scan groups into the chains;
                            # attention must wait for them (mkt8/mv8aug deps)
                            for _ in range(5):
                                if pending:
                                    pending.pop(0)()
                        elif actx is not None:
                            drain_values(us, actx, keep=2)
                            attn_scores(pstate["qf8"], 2 * ft, us)
                            attn_scores(pstate["qf8"], 2 * ft + 1, us)
                    if i == 0:
                        scan_a()
                    if i == 1:
                        while pending:
                            pending.pop(0)()
                        actx = attn_setup(pstate["feat16"])
                    if actx is not None:
                        drain_values(us, actx, keep=2)
                    for ft in range(4):
                        qf_chain(feat8, ft, qf8)
                        if i == 1:
                            attn_scores(pstate["qf8"], 2 * ft, us)
                            attn_scores(pstate["qf8"], 2 * ft + 1, us)
                            drain_values(us, actx, keep=2)
                        elif actx is not None:
                            drain_values(us, actx, keep=2)
                    if actx is not None:
                        drain_values(us, actx, keep=0)
                        drain_z(actx, keep=0)
                        attn_finish(actx, pch)
                    if last_enc:
                        # run this (final) chunk's attention here so the
                        # epilogue macro is only the z-assembly
                        actx2 = attn_setup(feat16)
                        us2 = []
                        for ft in range(4):
                            drain_values(us2, actx2, keep=2)
                            attn_scores(qf8, 2 * ft, us2)
                            attn_scores(qf8, 2 * ft + 1, us2)
                        state["us"] = us2
                        state["actx"] = actx2
                else:
                    # epilogue: finish the last chunk
                    drain_values(pstate["us"], pstate["actx"], keep=0)
                    drain_z(pstate["actx"], keep=0)
                    attn_finish(pstate["actx"], pch)

    nc.compile()
    return nc


def prep_inputs(inputs):
    """Host-side shard/layout prep. Returns per-core in_maps."""
    x = np.asarray(inputs["x"], dtype=np.float32)
    sx = np.asarray(inputs["support_x"], dtype=np.float32)
    sy = np.asarray(inputs["support_y"]).astype(np.int64)
    W_enc = np.asarray(inputs["W_enc"], dtype=np.float32)
    b_enc = np.asarray(inputs["b_enc"], dtype=np.float32)
    W_q = np.asarray(inputs["W_q"], dtype=np.float32)
    b_q = np.asarray(inputs["b_q"], dtype=np.float32)
    W_cls = np.asarray(inputs["W_cls"], dtype=np.float32)
    b_cls = np.asarray(inputs["b_cls"], dtype=np.float32)
    mem_keys = np.asarray(inputs["mem_keys"], dtype=np.float32)
    mem_values = np.asarray(inputs["mem_values"], dtype=np.float32)

    def pk(a, p=128):  # [K, N] -> [p, K/p, N] partition-major tiles
        k, n = a.shape
        return np.ascontiguousarray(a.reshape(k // p, p, n).transpose(1, 0, 2))

    # fold b_enc into the qf bias and the classifier bias (feat tiles are
    # produced without the encoder bias)
    bq_eff = b_enc @ W_q + b_q
    bcls_eff = b_cls + b_enc @ W_cls[:F]

    wenc_h = pk(_f16(W_enc))                     # [128, 16, F]
    sxt_h = pk(_f16(sx.T))                       # [128, 16, NS]
    wq8_h = pk(_f8(W_q))                         # [128, 4, F]
    wclst_h = pk(_f16(W_cls[:F]))                # [128, 4, NW]
    w2aug_h = np.zeros((DH + 1, H, NW), np.float16)
    w2aug_h[0] = _f16(bcls_eff / H)[None, :]
    w2aug_h[1:] = _f16(W_cls[F:]).reshape(H, DH, NW).transpose(1, 0, 2)
    mkt = np.ascontiguousarray(mem_keys.T)       # [F, M]
    mkt0_h = pk(mkt)
    mkt0b_h = pk(_f16(mkt * INV_SQRT_F))
    mvals_h = pk(_f16(mem_values))               # [128, 2, F]
    vals = np.zeros((NS, NW), np.float32)
    vals[np.arange(NS), sy] = 1.0
    valsb_h = pk(_f16(vals))                     # [128, 4, NW]
    benc_h = np.ascontiguousarray(b_enc.reshape(4, 128).T)
    bq_h = np.ascontiguousarray(bq_eff.reshape(4, 128).T)

    shared = dict(
        wenc=wenc_h, sxt=sxt_h, wq8=wq8_h, wclst=wclst_h, w2aug=w2aug_h,
        mkt0=mkt0_h, mkt0b=mkt0b_h, mvals=mvals_h, valsb=valsb_h,
        benc=benc_h, bq=bq_h,
    )
    in_maps = []
    for c in range(NCORES):
        m = dict(shared)
        # x^T fp16, chunk-major tiles: [128p, NCH, 16j, NB]
        xc = _f16(x[c * BC:(c + 1) * BC].T)      # [D_IN, BC]
        m["xs"] = np.ascontiguousarray(
            xc.reshape(16, 128, NCH, NB).transpose(1, 2, 0, 3))
        in_maps.append(m)
    return in_maps


def kernel_ex(inputs, trace=False, **kwargs):
    nc = build()
    in_maps = prep_inputs(inputs)
    res = run_bass_kernel_spmd(nc, in_maps, core_ids=list(range(NCORES)),
                               trace=trace, **kwargs)
    out = np.concatenate([r["y"] for r in res.results], axis=0)
    return out.astype(np.float32), res


def kernel(**inputs):
    out, _ = kernel_ex(inputs)
    return out


# revision 24
# speedup vs baseline: 1.0589x; 1.0589x over previous
"""Trainium2 Bass kernel for nn_MetaLearningWithMemory (v3).

Data-parallel over the query batch across 8 cores.  Key structure:
  - x is transposed + fp16-cast on the host (no on-device transposes, half
    the HBM traffic for x); wenc/x loads are split into quarter-tiles and
    interleaved so the first encoder chain starts ~3us into the kernel.
  - attention runs in the transposed [m, b] layout: scores^T via fp8
    matmuls, exp without accum_out, per-head softmax sums via a leading
    ones-column in the fp8 DoubleRow value matmul, and normalization
    deferred past tiny per-head classifier matmuls (b_cls folded in via the
    sum-row of the augmented classifier weights; b_enc folded into bq/bcls
    on the host).  The per-head z matmuls and sum-row transposes are
    emitted immediately after each head's value matmul; the final per-bt
    assembly is one broadcast multiply + strided reduce on DVE.
  - qf and the value stage use fp8 DoubleRow (2x PE rate).
  - emission is software-pipelined: attention of chunk N is interleaved
    into the encoder matmul chains of chunk N+1 (value matmuls deferred one
    slot behind their scores/exp), the scan's serial Jacobi groups are
    interleaved the same way, and the last chunk's attention rides inside
    its own macro so the epilogue is only the tiny z-assembly.
"""

from contextlib import ExitStack

import numpy as np

import concourse.bass as bass
import concourse.mybir as mybir
import concourse.tile as tile
from concourse import bacc
from concourse.bass_utils import run_bass_kernel_spmd

D_IN = 2048
F = 512
M = 256
NS = 512
H = 8
DH = 64
NW = 5
B = 16384
NCORES = 8
BC = B // NCORES          # 2048 batch rows per core
NB = 512                  # batch chunk
NCH = BC // NB            # 4 chunks
NITER = 2                 # Jacobi softmax passes (converges at 2)
INV_SQRT_F = float(F) ** -0.5
INV_SQRT_DH = float(DH) ** -0.5

f16 = mybir.dt.float16
f32 = mybir.dt.float32
f8 = mybir.dt.float8e4
FT = mybir.ActivationFunctionType
DR = mybir.MatmulPerfMode.DoubleRow
E4NP = mybir.dt.np(f8)


def _f16(a):
    return np.asarray(a, dtype=np.float32).astype(np.float16)


def _f8(a):
    return np.asarray(a, dtype=np.float32).astype(E4NP)


def build(stage="full", repeat=1, opts=None):
    opts = dict(opts or {})
    O = lambda k, d: opts.get(k, d)
    niter = O("niter", NITER)

    nc = bacc.Bacc("TRN2", target_bir_lowering=False)

    # ---- per-core external inputs (host-prepped layouts) ----
    xs = nc.dram_tensor("xs", [128, NCH, 16, NB], f16, kind="ExternalInput")
    wenc = nc.dram_tensor("wenc", [128, 16, F], f16, kind="ExternalInput")
    sxt = nc.dram_tensor("sxt", [128, 16, NS], f16, kind="ExternalInput")
    wq8 = nc.dram_tensor("wq8", [128, 4, F], f8, kind="ExternalInput")
    wclst = nc.dram_tensor("wclst", [128, 4, NW], f16, kind="ExternalInput")
    w2aug = nc.dram_tensor("w2aug", [65, H, NW], f16, kind="ExternalInput")
    mkt0 = nc.dram_tensor("mkt0", [128, 4, M], f32, kind="ExternalInput")
    mkt0b = nc.dram_tensor("mkt0b", [128, 4, M], f16, kind="ExternalInput")
    mvals = nc.dram_tensor("mvals", [128, 2, F], f16, kind="ExternalInput")
    valsb = nc.dram_tensor("valsb", [128, 4, NW], f16, kind="ExternalInput")
    benc = nc.dram_tensor("benc", [128, 4], f32, kind="ExternalInput")
    bq = nc.dram_tensor("bq", [128, 4], f32, kind="ExternalInput")
    y = nc.dram_tensor("y", [BC, NW], f32, kind="ExternalOutput")

    eye128 = nc.inline_tensor(np.eye(128, dtype=np.float16), name="eye128")
    # mask_su[s, t] = inv_sqrt_f if s < t else 0 (strict upper in [s, t])
    mask_np = np.triu(np.full((128, 128), INV_SQRT_F, np.float32), 1).astype(np.float16)
    mask_su = nc.inline_tensor(mask_np, name="mask_su")

    NMAC = NCH * repeat

    with tile.TileContext(nc) as tc:
        with ExitStack() as ctx:
            ep = ctx.enter_context
            const = ep(tc.tile_pool(name="const", bufs=1))
            persist = ep(tc.tile_pool(name="persist", bufs=1))
            xt_pool = ep(tc.tile_pool(name="xt", bufs=O("xt", 16)))
            ft_pool = ep(tc.tile_pool(name="ft16", bufs=O("ft16", 9)))
            f8_pool = ep(tc.tile_pool(name="ft8", bufs=O("ft8", 2)))
            qf_pool = ep(tc.tile_pool(name="qf8", bufs=O("qf8", 2)))
            u8_pool = ep(tc.tile_pool(name="u8", bufs=O("u8", 6)))
            mo_pool = ep(tc.tile_pool(name="mo", bufs=O("mo", 6)))
            r_pool = ep(tc.tile_pool(name="r", bufs=4))
            zs_pool = ep(tc.tile_pool(name="zs", bufs=4))
            tq_pool = ep(tc.tile_pool(name="tq", bufs=4))
            y_pool = ep(tc.tile_pool(name="ysb", bufs=2))
            w_pool = ep(tc.tile_pool(name="w", bufs=2))
            usb_pool = ep(tc.tile_pool(name="usb", bufs=4))
            rs_pool = ep(tc.tile_pool(name="rs", bufs=4))
            psA = ep(tc.tile_pool(name="psA", bufs=O("psA", 2), space="PSUM"))
            psS = ep(tc.tile_pool(name="psS", bufs=O("psS", 3), space="PSUM"))
            psV = ep(tc.tile_pool(name="psV", bufs=O("psV", 1), space="PSUM"))
            psZ = ep(tc.tile_pool(name="psZ", bufs=O("psZ", 1), space="PSUM"))

            # ============ constant + x loads, bandwidth-ordered ============
            # wenc and xt0 quarter-tiles interleaved so the chunk-0 encoder
            # can start after ~1/4 of each has landed; sxt next (scan); the
            # small constants ride behind.
            def load_xt_quarters(ch):
                parts = []
                for jp in range(4):
                    xt = xt_pool.tile([128, 4, NB], f16, tag="xt")
                    nc.gpsimd.dma_start(xt[:], xs[:, ch, 4 * jp:4 * jp + 4])
                    parts.append(xt)
                return parts

            wenc_t = []
            xt0_parts = []
            sxt_t = []
            for jp in range(4):
                wt = const.tile([128, 4, F], f16, tag="wenct", bufs=4,
                                name=f"wenc{jp}")
                nc.sync.dma_start(wt[:], wenc[:, 4 * jp:4 * jp + 4])
                wenc_t.append(wt)
                xt = xt_pool.tile([128, 4, NB], f16, tag="xt")
                nc.gpsimd.dma_start(xt[:], xs[:, 0, 4 * jp:4 * jp + 4])
                xt0_parts.append(xt)
                st = const.tile([128, 4, NS], f16, tag="sxtt", bufs=4,
                                name=f"sxt{jp}")
                nc.sync.dma_start(st[:], sxt[:, 4 * jp:4 * jp + 4])
                sxt_t.append(st)
            xts = [xt0_parts]
            if NMAC > 1:
                xts.append(load_xt_quarters(1))
            wq8_sb = const.tile([128, 4, F], f8)
            nc.sync.dma_start(wq8_sb[:], wq8[:])
            wclst_sb = const.tile([128, 4, NW], f16)
            nc.sync.dma_start(wclst_sb[:], wclst[:])
            w2aug_sb = const.tile([65, H, NW], f16)
            nc.sync.dma_start(w2aug_sb[:], w2aug[:])
            benc_sb = const.tile([128, 4], f32)
            nc.sync.dma_start(benc_sb[:], benc[:])
            bq_sb = const.tile([128, 4], f32)
            nc.sync.dma_start(bq_sb[:], bq[:])
            eye128_sb = const.tile([128, 128], f16)
            nc.sync.dma_start(eye128_sb[:], eye128[:])
            mask_sb = const.tile([128, 128], f16)
            nc.sync.dma_start(mask_sb[:], mask_su[:])
            mkt0b_sb = const.tile([128, 4, M], f16)
            nc.sync.dma_start(mkt0b_sb[:], mkt0b[:])
            for ch in range(2, min(NCH, NMAC)):
                xts.append(load_xt_quarters(ch))
            mkt0_sb = const.tile([128, 4, M], f32)
            nc.sync.dma_start(mkt0_sb[:], mkt0[:])
            mv_sb = const.tile([128, 2, F], f16)
            nc.sync.dma_start(mv_sb[:], mvals[:])
            vals_sb = const.tile([128, 4, NW], f16)
            nc.sync.dma_start(vals_sb[:], valsb[:])

            # persistent scan results
            st_bf = persist.tile([128, 4, NS], f16, name="st_bf")
            s_sb = persist.tile([128, 4, F], f16, name="s_sb")
            g_sb = persist.tile([128, 4, NS], f16, name="g_sb")
            base_sb = persist.tile([128, 4, M], f32, name="base_sb")
            mkt8 = persist.tile([128, 4, M], f8, name="mkt8")
            mv_nat = persist.tile([128, 2, F], f16, name="mv_nat")
            mv8aug = persist.tile([128, 2, H, 80], f8, name="mv8aug")

            # ================= scan part A (batch-shaped) =================
            def scan_a():
                # S^T [f, t] = W_enc^T Sx^T; + b_enc
                for ft in range(4):
                    ps = psA.tile([128, NB], f32, tag="psA")
                    for j in range(16):
                        nc.tensor.matmul(
                            ps[:], wenc_t[j // 4][:, j % 4, ft * 128:(ft + 1) * 128],
                            sxt_t[j // 4][:, j % 4, :],
                            start=(j == 0), stop=(j == 15),
                        )
                    nc.scalar.activation(st_bf[:, ft, :], ps[:], FT.Identity,
                                         bias=benc_sb[:, ft:ft + 1])
                # S natural [t, f] via PE transpose of S^T
                for tt in range(4):
                    pt = psS.tile([128, NB], f16, tag="psS")
                    for ft in range(4):
                        nc.tensor.transpose(
                            pt[:, ft * 128:(ft + 1) * 128],
                            st_bf[:, ft, tt * 128:(tt + 1) * 128], eye128_sb[:])
                    nc.vector.tensor_copy(s_sb[:, tt, :], pt[:])
                # G[s, t] = (S S^T)/sqrt(F); diag blocks masked strict-upper
                for ks in range(4):
                    ps = psA.tile([128, NB], f32, tag="psA")
                    for kf in range(4):
                        nc.tensor.matmul(
                            ps[:], st_bf[:, kf, ks * 128:(ks + 1) * 128],
                            st_bf[:, kf, :], start=(kf == 0), stop=(kf == 3),
                        )
                    for tt in range(4):
                        dst = g_sb[:, ks, tt * 128:(tt + 1) * 128]
                        src = ps[:, tt * 128:(tt + 1) * 128]
                        if tt == ks:
                            nc.vector.tensor_mul(dst, src, mask_sb[:])
                        elif tt > ks:
                            nc.vector.tensor_scalar_mul(dst, src, INV_SQRT_F)
                # base[t, m] = S @ mem_keys^T / sqrt(F)
                for tt in range(4):
                    pl = psS.tile([128, M], f32, tag="psS")
                    for kf in range(4):
                        nc.tensor.matmul(
                            pl[:], st_bf[:, kf, tt * 128:(tt + 1) * 128],
                            mkt0b_sb[:, kf, :], start=(kf == 0), stop=(kf == 3),
                        )
                    nc.scalar.copy(base_sb[:, tt, :], pl[:])

            # ============ scan part B: serial Jacobi groups ============
            # returned as closures; interleaved into macro 1's encoder chains
            scan_groups = []
            w_tiles = [None]

            def jac0(tt):
                def g():
                    if tt == 0:
                        w_tiles[0] = w_pool.tile([128, 4, M], f16, tag="wt",
                                                 name="w_it0")
                    w_cur = w_tiles[0]
                    u = usb_pool.tile([128, M], f32, tag="usb")
                    ssum = rs_pool.tile([128, 1], f32, tag="scol")
                    nc.scalar.activation(u[:], base_sb[:, tt, :], FT.Exp,
                                         accum_out=ssum[:])
                    rcol = rs_pool.tile([128, 1], f32, tag="scol")
                    nc.vector.reciprocal(rcol[:], ssum[:])
                    nc.vector.tensor_scalar_mul(w_cur[:, tt, :], u[:], rcol[:])
                return g

            def jac(it, tt):
                def g():
                    w_cur = w_tiles[0]
                    if tt == 0:
                        w_tiles.append(w_pool.tile([128, 4, M], f16, tag="wt",
                                                   name=f"w_it{it}"))
                    w_new = w_tiles[-1]
                    pl = psS.tile([128, M], f32, tag="psS")
                    for ks in range(tt + 1):
                        nc.tensor.matmul(
                            pl[:], g_sb[:, ks, tt * 128:(tt + 1) * 128],
                            w_cur[:, ks, :], start=(ks == 0), stop=(ks == tt),
                        )
                    nc.vector.tensor_add(pl[:], pl[:], base_sb[:, tt, :])
                    u = usb_pool.tile([128, M], f32, tag="usb")
                    ssum = rs_pool.tile([128, 1], f32, tag="scol")
                    nc.scalar.activation(u[:], pl[:], FT.Exp, accum_out=ssum[:])
                    rcol = rs_pool.tile([128, 1], f32, tag="scol")
                    nc.vector.reciprocal(rcol[:], ssum[:])
                    nc.vector.tensor_scalar_mul(w_new[:, tt, :], u[:], rcol[:])
                    if tt == 3:
                        w_tiles[0] = w_new
                return g

            def mk_group(ft):
                def g():
                    w_cur = w_tiles[0]
                    pl = psS.tile([128, M], f32, tag="psS")
                    for tt in range(4):
                        nc.tensor.matmul(
                            pl[:], s_sb[:, tt, ft * 128:(ft + 1) * 128],
                            w_cur[:, tt, :], start=(tt == 0), stop=(tt == 3),
                        )
                    nc.vector.tensor_add(mkt8[:, ft, :], pl[:],
                                         mkt0_sb[:, ft, :])
                return g

            def mv_group(mt):
                def g():
                    w_cur = w_tiles[0]
                    nc.vector.tensor_copy(mv_nat[:, mt, NW:], mv_sb[:, mt, NW:])
                    pl = psS.tile([128, M], f32, tag="psS")
                    for tt in range(4):
                        nc.tensor.matmul(
                            pl[:, 0:NW], w_cur[:, tt, mt * 128:(mt + 1) * 128],
                            vals_sb[:, tt, :], start=(tt == 0), stop=(tt == 3),
                        )
                    nc.vector.tensor_add(mv_nat[:, mt, 0:NW], pl[:, 0:NW],
                                         mv_sb[:, mt, 0:NW])
                return g

            def mv8_group():
                def g():
                    # aug layout per head: col 0 = ones, cols 1..65 = values
                    nc.vector.memset(mv8aug[:, :, :, 0:1], 1.0)
                    nc.vector.tensor_copy(
                        mv8aug[:, :, :, 1:DH + 1],
                        mv_nat[:].rearrange("p mt (h d) -> p mt h d", d=DH))
                return g

            for tt in range(4):
                scan_groups.append(jac0(tt))
            for it in range(1, niter):
                for tt in range(4):
                    scan_groups.append(jac(it, tt))
            for ft in range(4):
                scan_groups.append(mk_group(ft))
            for mt in range(2):
                scan_groups.append(mv_group(mt))
            scan_groups.append(mv8_group())

            # ================= main pipeline stages =================
            def feat_chain(xt_parts, ft, feat16, feat8, act_light=False):
                ps = psA.tile([128, NB], f32, tag="psA")
                for j in range(16):
                    nc.tensor.matmul(
                        ps[:], wenc_t[j // 4][:, j % 4, ft * 128:(ft + 1) * 128],
                        xt_parts[j // 4][:, j % 4, :],
                        start=(j == 0), stop=(j == 15),
                    )
                f16t = ft_pool.tile([128, NB], f16, tag="ft16")
                if act_light:
                    nc.vector.tensor_copy(f16t[:], ps[:])
                else:
                    nc.scalar.copy(f16t[:], ps[:])
                feat16.append(f16t)
                nc.vector.tensor_copy(feat8[:, ft, :], ps[:])

            def qf_chain(feat8, ft, qf8):
                ps = psA.tile([128, NB], f32, tag="psA")
                for u2 in range(2):
                    nc.tensor.matmul(
                        ps[:], wq8_sb[:, 2 * u2:2 * u2 + 2,
                                      ft * 128:(ft + 1) * 128],
                        feat8[:, 2 * u2:2 * u2 + 2, :],
                        start=(u2 == 0), stop=(u2 == 1), perf_mode=DR,
                    )
                nc.vector.tensor_scalar_add(qf8[:, ft, :], ps[:],
                                            bq_sb[:, ft:ft + 1])

            def attn_setup(feat16):
                # logits1 accumulation + per-bt z column groups, one bank
                pz4 = psZ.tile([128, 4, 48], f32, tag="pz")
                pr4 = psV.tile([128, 4, H, 2], f16, tag="pr", bufs=1)
                for bt in range(4):
                    bsl = slice(bt * 128, (bt + 1) * 128)
                    for k in range(4):
                        nc.tensor.matmul(pz4[:, bt, 40:45],
                                         feat16[k][:, bsl], wclst_sb[:, k, :],
                                         start=(k == 0), stop=(k == 3))
                return dict(pz4=pz4, pr4=pr4, zq=[])

            def attn_scores(qf8, h, us):
                kf, p0 = h // 2, 64 * (h % 2)
                u8 = u8_pool.tile([128, 2, NB], f8, tag="u8")
                for mj in range(2):
                    ps = psS.tile([128, NB], f32, tag="psS")
                    nc.tensor.matmul(
                        ps[:],
                        mkt8[p0:p0 + 64, kf, mj * 128:(mj + 1) * 128],
                        qf8[p0:p0 + 64, kf, :], start=True, stop=True,
                    )
                    nc.scalar.activation(u8[:, mj, :], ps[:], FT.Exp,
                                         scale=INV_SQRT_DH)
                us.append((h, u8))

            def attn_value(h, u8, actx):
                pv = psV.tile([65, NB], f32, tag="psV")
                nc.tensor.matmul(pv[:], mv8aug[:, :, h, 0:DH + 1], u8[:],
                                 start=True, stop=True, perf_mode=DR)
                mo = mo_pool.tile([65, NB], f16, tag="mo")
                if h % 2 == 0 and not actx.get("light"):
                    nc.scalar.copy(mo[:], pv[:])
                else:
                    nc.vector.tensor_copy(mo[:], pv[:])
                actx["zq"].append((h, mo))

            def emit_z(h, mo, actx):
                pz4, pr4 = actx["pz4"], actx["pr4"]
                for bt in range(4):
                    bsl = slice(bt * 128, (bt + 1) * 128)
                    nc.tensor.transpose(pr4[:, bt, h, 0:1], mo[0:1, bsl],
                                        eye128_sb[0:1, 0:1])
                    nc.tensor.matmul(pz4[:, bt, 5 * h:5 * h + 5], mo[:, bsl],
                                     w2aug_sb[:, h, :], start=True, stop=True)

            def drain_z(actx, keep=0):
                zq = actx["zq"]
                while len(zq) > keep:
                    h, mo = zq.pop(0)
                    emit_z(h, mo, actx)

            def drain_values(us, actx, keep=0):
                while len(us) > keep:
                    h, u8 = us.pop(0)
                    attn_value(h, u8, actx)
                    drain_z(actx, keep=1)

            def attn_finish(actx, pch):
                # stage-major emission: consecutive DVE ops are independent
                # so they pipeline instead of paying the dep-chain latency
                pz4, pr4 = actx["pz4"], actx["pr4"]
                ysb = y_pool.tile([128, NCH, NW], f32, tag="ysb")
                rs, zss, tqs = [], [], []
                for bt in range(4):
                    r = r_pool.tile([128, 1, H], f32, tag="r")
                    nc.vector.reciprocal(
                        r[:],
                        pr4[:, bt, :, 0:1].rearrange("p h one -> p one h"))
                    rs.append(r)
                for bt in range(4):
                    zs = zs_pool.tile([128, NW, H], f32, tag="zs")
                    zv, rv = bass.broadcast_tensor_aps(
                        pz4[:, bt, 0:40].rearrange("p (h c) -> p c h", c=NW),
                        rs[bt][:])
                    nc.vector.tensor_mul(zs[:], zv, rv)
                    zss.append(zs)
                for bt in range(4):
                    tq = tq_pool.tile([128, NW], f32, tag="tq")
                    nc.vector.tensor_reduce(tq[:], zss[bt][:],
                                            axis=mybir.AxisListType.X,
                                            op=mybir.AluOpType.add)
                    tqs.append(tq)
                for bt in range(4):
                    nc.vector.tensor_add(ysb[:, bt, :], tqs[bt][:],
                                         pz4[:, bt, 40:45])
                nc.sync.dma_start(
                    y[pch * NB:(pch + 1) * NB, :].rearrange(
                        "(bt p) c -> p bt c", p=128),
                    ysb[:])

            # ================= software-pipelined macro loop =================
            # macro i: encoder+qf for chunk i; attention+z+y for chunk i-1
            # interleaved into the chains.  Macro 1 runs chunk 0's attention
            # in its qf phase (the feat phase carries the scan's serial
            # groups); the last encoder macro additionally runs its own
            # chunk's attention in its qf phase so the epilogue is only the
            # z-assembly.
            state = {}
            pending = list(scan_groups)

            for i in range(NMAC + 1):
                enc = i < NMAC
                last_enc = i == NMAC - 1
                ch = i % NCH
                pch = (i - 1) % NCH
                pstate = state
                state = {}
                us = []
                actx = None
                if i >= 2 and "us" not in pstate:
                    actx = attn_setup(pstate["feat16"])
                if enc:
                    xt_parts = xts[ch]
                    feat16 = []
                    feat8 = f8_pool.tile([128, 4, NB], f8, tag="ft8")
                    qf8 = qf_pool.tile([128, 4, NB], f8, tag="qf8")
                    state = dict(feat16=feat16, qf8=qf8)
                    for ft in range(4):
                        feat_chain(xt_parts, ft, feat16, feat8,
                                   act_light=(last_enc and O("light", 1) > 0))
                        if i == 1:
                            # interleave serial scan groups into the chains;
                            # attention must wait for them (mkt8/mv8aug deps)
                            for _ in range(5):
                                if pending:
                                    pending.pop(0)()
                        elif actx is not None:
                            drain_values(us, actx, keep=2)
                            attn_scores(pstate["qf8"], 2 * ft, us)
                            attn_scores(pstate["qf8"], 2 * ft + 1, us)
                    if i == 0:
                        scan_a()
                    if i == 1:
                        while pending:
                            pending.pop(0)()
                        actx = attn_setup(pstate["feat16"])
                    if actx is not None:
                        drain_values(us, actx, keep=2)
                    for ft in range(4):
                        qf_chain(feat8, ft, qf8)
                        if i == 1:
                            attn_scores(pstate["qf8"], 2 * ft, us)
                            attn_scores(pstate["qf8"], 2 * ft + 1, us)
                            drain_values(us, actx, keep=2)
                        elif actx is not None:
                            drain_values(us, actx, keep=2)
                    if actx is not None:
                        drain_values(us, actx, keep=0)
                        drain_z(actx, keep=0)
                        attn_finish(actx, pch)
                    if last_enc:
                        # run this (final) chunk's attention here so the
                        # epilogue macro is only the z-assembly
                        actx2 = attn_setup(feat16)
                        actx2["light"] = O("light", 1) > 0
                        us2 = []
                        for ft in range(4):
                            drain_values(us2, actx2, keep=2)
                            attn_scores(qf8, 2 * ft, us2)
                            attn_scores(qf8, 2 * ft + 1, us2)
                        state["us"] = us2
                        state["actx"] = actx2
                else:
                    # epilogue: finish the last chunk
                    drain_values(pstate["us"], pstate["actx"], keep=0)
                    drain_z(pstate["actx"], keep=0)
                    attn_finish(pstate["actx"], pch)

    nc.compile()
    return nc


def prep_inputs(inputs):
    """Host-side shard/layout prep. Returns per-core in_maps."""
    x = np.asarray(inputs["x"], dtype=np.float32)
    sx = np.asarray(inputs["support_x"], dtype=np.float32)
    sy = np.asarray(inputs["support_y"]).astype(np.int64)
    W_enc = np.asarray(inputs["W_enc"], dtype=np.float32)
    b_enc = np.asarray(inputs["b_enc"], dtype=np.float32)
    W_q = np.asarray(inputs["W_q"], dtype=np.float32)
    b_q = np.asarray(inputs["b_q"], dtype=np.float32)
    W_cls = np.asarray(inputs["W_cls"], dtype=np.float32)
    b_cls = np.asarray(inputs["b_cls"], dtype=np.float32)
    mem_keys = np.asarray(inputs["mem_keys"], dtype=np.float32)
    mem_values = np.asarray(inputs["mem_values"], dtype=np.float32)

    def pk(a, p=128):  # [K, N] -> [p, K/p, N] partition-major tiles
        k, n = a.shape
        return np.ascontiguousarray(a.reshape(k // p, p, n).transpose(1, 0, 2))

    # fold b_enc into the qf bias and the classifier bias (feat tiles are
    # produced without the encoder bias)
    bq_eff = b_enc @ W_q + b_q
    bcls_eff = b_cls + b_enc @ W_cls[:F]

    wenc_h = pk(_f16(W_enc))                     # [128, 16, F]
    sxt_h = pk(_f16(sx.T))                       # [128, 16, NS]
    wq8_h = pk(_f8(W_q))                         # [128, 4, F]
    wclst_h = pk(_f16(W_cls[:F]))                # [128, 4, NW]
    w2aug_h = np.zeros((DH + 1, H, NW), np.float16)
    w2aug_h[0] = _f16(bcls_eff / H)[None, :]
    w2aug_h[1:] = _f16(W_cls[F:]).reshape(H, DH, NW).transpose(1, 0, 2)
    mkt = np.ascontiguousarray(mem_keys.T)       # [F, M]
    mkt0_h = pk(mkt)
    mkt0b_h = pk(_f16(mkt * INV_SQRT_F))
    mvals_h = pk(_f16(mem_values))               # [128, 2, F]
    vals = np.zeros((NS, NW), np.float32)
    vals[np.arange(NS), sy] = 1.0
    valsb_h = pk(_f16(vals))                     # [128, 4, NW]
    benc_h = np.ascontiguousarray(b_enc.reshape(4, 128).T)
    bq_h = np.ascontiguousarray(bq_eff.reshape(4, 128).T)

    shared = dict(
        wenc=wenc_h, sxt=sxt_h, wq8=wq8_h, wclst=wclst_h, w2aug=w2aug_h,
        mkt0=mkt0_h, mkt0b=mkt0b_h, mvals=mvals_h, valsb=valsb_h,
        benc=benc_h, bq=bq_h,
    )
    in_maps = []
    for c in range(NCORES):
        m = dict(shared)
        # x^T fp16, chunk-major tiles: [128p, NCH, 16j, NB]
        xc = _f16(x[c * BC:(c + 1) * BC].T)      # [D_IN, BC]
        m["xs"] = np.ascontiguousarray(
            xc.reshape(16, 128, NCH, NB).transpose(1, 2, 0, 3))
        in_maps.append(m)
    return in_maps


def kernel_ex(inputs, trace=False, **kwargs):
    nc = build()
    in_maps = prep_inputs(inputs)
    res = run_bass_kernel_spmd(nc, in_maps, core_ids=list(range(NCORES)),
                               trace=trace, **kwargs)
    out = np.concatenate([r["y"] for r in res.results], axis=0)
    return out.astype(np.float32), res


def kernel(**inputs):
    out, _ = kernel_ex(inputs)
    return out


# revision 25
# speedup vs baseline: 1.2623x; 1.1921x over previous
"""Trainium2 Bass kernel for nn_MetaLearningWithMemory (v3).

Data-parallel over the query batch across 8 cores.  Key structure:
  - x is transposed + fp16-cast on the host (no on-device transposes, half
    the HBM traffic for x); wenc/x loads are split into quarter-tiles and
    interleaved so the first encoder chain starts ~3us into the kernel.
  - attention runs in the transposed [m, b] layout: scores^T via fp8
    matmuls, exp without accum_out, per-head softmax sums via a leading
    ones-column in the fp8 DoubleRow value matmul, and normalization
    deferred past tiny per-head classifier matmuls (b_cls folded in via the
    sum-row of the augmented classifier weights; b_enc folded into bq/bcls
    on the host).  The per-head z matmuls and sum-row transposes are
    emitted immediately after each head's value matmul; the final per-bt
    assembly is one broadcast multiply + strided reduce on DVE.
  - qf and the value stage use fp8 DoubleRow (2x PE rate).
  - emission is software-pipelined: attention of chunk N is interleaved
    into the encoder matmul chains of chunk N+1 (value matmuls deferred one
    slot behind their scores/exp), the scan's serial Jacobi groups are
    interleaved the same way, and the last chunk's attention rides inside
    its own macro so the epilogue is only the tiny z-assembly.
"""

from contextlib import ExitStack

import numpy as np

import concourse.bass as bass
import concourse.mybir as mybir
import concourse.tile as tile
from concourse import bacc
from concourse.bass_utils import run_bass_kernel_spmd

D_IN = 2048
F = 512
M = 256
NS = 512
H = 8
DH = 64
NW = 5
B = 16384
NCORES = 8
BC = B // NCORES          # 2048 batch rows per core
NB = 512                  # batch chunk
NCH = BC // NB            # 4 chunks
NITER = 2                 # Jacobi softmax passes (converges at 2)
INV_SQRT_F = float(F) ** -0.5
INV_SQRT_DH = float(DH) ** -0.5

f16 = mybir.dt.float16
f32 = mybir.dt.float32
f8 = mybir.dt.float8e4
FT = mybir.ActivationFunctionType
DR = mybir.MatmulPerfMode.DoubleRow
E4NP = mybir.dt.np(f8)


def _f16(a):
    return np.asarray(a, dtype=np.float32).astype(np.float16)


def _f8(a):
    return np.asarray(a, dtype=np.float32).astype(E4NP)


def build(stage="full", repeat=1, opts=None):
    opts = dict(opts or {})
    O = lambda k, d: opts.get(k, d)
    niter = O("niter", NITER)

    nc = bacc.Bacc("TRN2", target_bir_lowering=False)

    # ---- per-core external inputs (host-prepped layouts) ----
    xs = nc.dram_tensor("xs", [128, NCH, 16, NB], f16, kind="ExternalInput")
    wenc = nc.dram_tensor("wenc", [128, 16, F], f16, kind="ExternalInput")
    sxt = nc.dram_tensor("sxt", [128, 16, NS], f8, kind="ExternalInput")
    wenc8 = nc.dram_tensor("wenc8", [128, 16, F], f8, kind="ExternalInput")
    wq8 = nc.dram_tensor("wq8", [128, 4, F], f8, kind="ExternalInput")
    wclst = nc.dram_tensor("wclst", [128, 4, NW], f16, kind="ExternalInput")
    w2aug = nc.dram_tensor("w2aug", [65, H, NW], f16, kind="ExternalInput")
    mkt0 = nc.dram_tensor("mkt0", [128, 4, M], f32, kind="ExternalInput")
    mkt0b = nc.dram_tensor("mkt0b", [128, 4, M], f8, kind="ExternalInput")
    mvals = nc.dram_tensor("mvals", [128, 2, F], f16, kind="ExternalInput")
    valsb = nc.dram_tensor("valsb", [128, 4, NW], f16, kind="ExternalInput")
    benc = nc.dram_tensor("benc", [128, 4], f32, kind="ExternalInput")
    bq = nc.dram_tensor("bq", [128, 4], f32, kind="ExternalInput")
    y = nc.dram_tensor("y", [BC, NW], f32, kind="ExternalOutput")

    eye128 = nc.inline_tensor(np.eye(128, dtype=np.float16), name="eye128")
    # mask_su[s, t] = inv_sqrt_f if s < t else 0 (strict upper in [s, t])
    mask_np = np.triu(np.full((128, 128), INV_SQRT_F, np.float32), 1).astype(np.float16)
    mask_su = nc.inline_tensor(mask_np, name="mask_su")

    NMAC = NCH * repeat

    with tile.TileContext(nc) as tc:
        with ExitStack() as ctx:
            ep = ctx.enter_context
            const = ep(tc.tile_pool(name="const", bufs=1))
            persist = ep(tc.tile_pool(name="persist", bufs=1))
            xt_pool = ep(tc.tile_pool(name="xt", bufs=O("xt", 16)))
            ft_pool = ep(tc.tile_pool(name="ft16", bufs=O("ft16", 9)))
            f8_pool = ep(tc.tile_pool(name="ft8", bufs=O("ft8", 2)))
            qf_pool = ep(tc.tile_pool(name="qf8", bufs=O("qf8", 2)))
            u8_pool = ep(tc.tile_pool(name="u8", bufs=O("u8", 6)))
            mo_pool = ep(tc.tile_pool(name="mo", bufs=O("mo", 6)))
            r_pool = ep(tc.tile_pool(name="r", bufs=4))
            zs_pool = ep(tc.tile_pool(name="zs", bufs=4))
            tq_pool = ep(tc.tile_pool(name="tq", bufs=4))
            y_pool = ep(tc.tile_pool(name="ysb", bufs=2))
            w_pool = ep(tc.tile_pool(name="w", bufs=2))
            usb_pool = ep(tc.tile_pool(name="usb", bufs=4))
            rs_pool = ep(tc.tile_pool(name="rs", bufs=4))
            psA = ep(tc.tile_pool(name="psA", bufs=O("psA", 2), space="PSUM"))
            psS = ep(tc.tile_pool(name="psS", bufs=O("psS", 3), space="PSUM"))
            psV = ep(tc.tile_pool(name="psV", bufs=O("psV", 1), space="PSUM"))
            psZ = ep(tc.tile_pool(name="psZ", bufs=O("psZ", 1), space="PSUM"))

            # ============ constant + x loads, bandwidth-ordered ============
            # wenc and xt0 quarter-tiles interleaved so the chunk-0 encoder
            # can start after ~1/4 of each has landed; sxt next (scan); the
            # small constants ride behind.
            def load_xt_quarters(ch):
                parts = []
                for jp in range(4):
                    xt = xt_pool.tile([128, 4, NB], f16, tag="xt")
                    nc.gpsimd.dma_start(xt[:], xs[:, ch, 4 * jp:4 * jp + 4])
                    parts.append(xt)
                return parts

            wenc_t = []
            xt0_parts = []
            for jp in range(4):
                wt = const.tile([128, 4, F], f16, tag="wenct", bufs=4,
                                name=f"wenc{jp}")
                nc.sync.dma_start(wt[:], wenc[:, 4 * jp:4 * jp + 4])
                wenc_t.append(wt)
                xt = xt_pool.tile([128, 4, NB], f16, tag="xt")
                nc.gpsimd.dma_start(xt[:], xs[:, 0, 4 * jp:4 * jp + 4])
                xt0_parts.append(xt)
            sxt8_sb = const.tile([128, 16, NS], f8)
            nc.sync.dma_start(sxt8_sb[:], sxt[:])
            wenc8_sb = const.tile([128, 16, F], f8)
            nc.sync.dma_start(wenc8_sb[:], wenc8[:])
            xts = [xt0_parts]
            if NMAC > 1:
                xts.append(load_xt_quarters(1))
            wq8_sb = const.tile([128, 4, F], f8)
            nc.sync.dma_start(wq8_sb[:], wq8[:])
            wclst_sb = const.tile([128, 4, NW], f16)
            nc.sync.dma_start(wclst_sb[:], wclst[:])
            w2aug_sb = const.tile([65, H, NW], f16)
            nc.sync.dma_start(w2aug_sb[:], w2aug[:])
            benc_sb = const.tile([128, 4], f32)
            nc.sync.dma_start(benc_sb[:], benc[:])
            bq_sb = const.tile([128, 4], f32)
            nc.sync.dma_start(bq_sb[:], bq[:])
            eye128_sb = const.tile([128, 128], f16)
            nc.sync.dma_start(eye128_sb[:], eye128[:])
            mask_sb = const.tile([128, 128], f16)
            nc.sync.dma_start(mask_sb[:], mask_su[:])
            mkt0b_sb = const.tile([128, 4, M], f8)
            nc.sync.dma_start(mkt0b_sb[:], mkt0b[:])
            for ch in range(2, min(NCH, NMAC)):
                xts.append(load_xt_quarters(ch))
            mkt0_sb = const.tile([128, 4, M], f32)
            nc.sync.dma_start(mkt0_sb[:], mkt0[:])
            mv_sb = const.tile([128, 2, F], f16)
            nc.sync.dma_start(mv_sb[:], mvals[:])
            vals_sb = const.tile([128, 4, NW], f16)
            nc.sync.dma_start(vals_sb[:], valsb[:])

            # persistent scan results
            st_bf = persist.tile([128, 4, NS], f16, name="st_bf")
            st8 = persist.tile([128, 4, NS], f8, name="st8")
            s_sb = persist.tile([128, 4, F], f16, name="s_sb")
            g_sb = persist.tile([128, 4, NS], f16, name="g_sb")
            base_sb = persist.tile([128, 4, M], f32, name="base_sb")
            mkt8 = persist.tile([128, 4, M], f8, name="mkt8")
            mv_nat = persist.tile([128, 2, F], f16, name="mv_nat")
            mv8aug = persist.tile([128, 2, H, 80], f8, name="mv8aug")

            # ================= scan part A (batch-shaped) =================
            def scan_a():
                # S^T [f, t] = W_enc^T Sx^T via fp8 DoubleRow; + b_enc
                for ft in range(4):
                    ps = psA.tile([128, NB], f32, tag="psA")
                    for j2 in range(8):
                        nc.tensor.matmul(
                            ps[:],
                            wenc8_sb[:, 2 * j2:2 * j2 + 2,
                                     ft * 128:(ft + 1) * 128],
                            sxt8_sb[:, 2 * j2:2 * j2 + 2, :],
                            start=(j2 == 0), stop=(j2 == 7), perf_mode=DR,
                        )
                    nc.scalar.activation(st_bf[:, ft, :], ps[:], FT.Identity,
                                         bias=benc_sb[:, ft:ft + 1])
                    nc.vector.tensor_scalar_add(st8[:, ft, :], ps[:],
                                                benc_sb[:, ft:ft + 1])
                # S natural [t, f] via PE transpose of S^T
                for tt in range(4):
                    pt = psS.tile([128, NB], f16, tag="psS")
                    for ft in range(4):
                        nc.tensor.transpose(
                            pt[:, ft * 128:(ft + 1) * 128],
                            st_bf[:, ft, tt * 128:(tt + 1) * 128], eye128_sb[:])
                    nc.vector.tensor_copy(s_sb[:, tt, :], pt[:])
                # G[s, t] = (S S^T)/sqrt(F); diag blocks masked strict-upper
                for ks in range(4):
                    ps = psA.tile([128, NB], f32, tag="psA")
                    for kf in range(4):
                        nc.tensor.matmul(
                            ps[:], st_bf[:, kf, ks * 128:(ks + 1) * 128],
                            st_bf[:, kf, :], start=(kf == 0), stop=(kf == 3),
                        )
                    for tt in range(4):
                        dst = g_sb[:, ks, tt * 128:(tt + 1) * 128]
                        src = ps[:, tt * 128:(tt + 1) * 128]
                        if tt == ks:
                            nc.vector.tensor_mul(dst, src, mask_sb[:])
                        elif tt > ks:
                            nc.vector.tensor_scalar_mul(dst, src, INV_SQRT_F)
                # base[t, m] = S @ mem_keys^T / sqrt(F), fp8 DoubleRow
                for tt in range(4):
                    pl = psS.tile([128, M], f32, tag="psS")
                    for u2 in range(2):
                        nc.tensor.matmul(
                            pl[:],
                            st8[:, 2 * u2:2 * u2 + 2,
                                tt * 128:(tt + 1) * 128],
                            mkt0b_sb[:, 2 * u2:2 * u2 + 2, :],
                            start=(u2 == 0), stop=(u2 == 1), perf_mode=DR,
                        )
                    nc.scalar.copy(base_sb[:, tt, :], pl[:])

            # ============ scan part B: serial Jacobi groups ============
            # returned as closures; interleaved into macro 1's encoder chains
            scan_groups = []
            w_tiles = [None]

            def jac0(tt):
                def g():
                    if tt == 0:
                        w_tiles[0] = w_pool.tile([128, 4, M], f16, tag="wt",
                                                 name="w_it0")
                    w_cur = w_tiles[0]
                    u = usb_pool.tile([128, M], f32, tag="usb")
                    ssum = rs_pool.tile([128, 1], f32, tag="scol")
                    nc.scalar.activation(u[:], base_sb[:, tt, :], FT.Exp,
                                         accum_out=ssum[:])
                    rcol = rs_pool.tile([128, 1], f32, tag="scol")
                    nc.vector.reciprocal(rcol[:], ssum[:])
                    nc.vector.tensor_scalar_mul(w_cur[:, tt, :], u[:], rcol[:])
                return g

            def jac(it, tt):
                def g():
                    w_cur = w_tiles[0]
                    if tt == 0:
                        w_tiles.append(w_pool.tile([128, 4, M], f16, tag="wt",
                                                   name=f"w_it{it}"))
                    w_new = w_tiles[-1]
                    pl = psS.tile([128, M], f32, tag="psS")
                    for ks in range(tt + 1):
                        nc.tensor.matmul(
                            pl[:], g_sb[:, ks, tt * 128:(tt + 1) * 128],
                            w_cur[:, ks, :], start=(ks == 0), stop=(ks == tt),
                        )
                    nc.vector.tensor_add(pl[:], pl[:], base_sb[:, tt, :])
                    u = usb_pool.tile([128, M], f32, tag="usb")
                    ssum = rs_pool.tile([128, 1], f32, tag="scol")
                    nc.scalar.activation(u[:], pl[:], FT.Exp, accum_out=ssum[:])
                    rcol = rs_pool.tile([128, 1], f32, tag="scol")
                    nc.vector.reciprocal(rcol[:], ssum[:])
                    nc.vector.tensor_scalar_mul(w_new[:, tt, :], u[:], rcol[:])
                    if tt == 3:
                        w_tiles[0] = w_new
                return g

            def mk_group(ft):
                def g():
                    w_cur = w_tiles[0]
                    pl = psS.tile([128, M], f32, tag="psS")
                    for tt in range(4):
                        nc.tensor.matmul(
                            pl[:], s_sb[:, tt, ft * 128:(ft + 1) * 128],
                            w_cur[:, tt, :], start=(tt == 0), stop=(tt == 3),
                        )
                    nc.vector.tensor_add(mkt8[:, ft, :], pl[:],
                                         mkt0_sb[:, ft, :])
                return g

            def mv_group(mt):
                def g():
                    w_cur = w_tiles[0]
                    nc.vector.tensor_copy(mv_nat[:, mt, NW:], mv_sb[:, mt, NW:])
                    pl = psS.tile([128, M], f32, tag="psS")
                    for tt in range(4):
                        nc.tensor.matmul(
                            pl[:, 0:NW], w_cur[:, tt, mt * 128:(mt + 1) * 128],
                            vals_sb[:, tt, :], start=(tt == 0), stop=(tt == 3),
                        )
                    nc.vector.tensor_add(mv_nat[:, mt, 0:NW], pl[:, 0:NW],
                                         mv_sb[:, mt, 0:NW])
                return g

            def mv8_group():
                def g():
                    # aug layout per head: col 0 = ones, cols 1..65 = values
                    nc.vector.memset(mv8aug[:, :, :, 0:1], 1.0)
                    nc.vector.tensor_copy(
                        mv8aug[:, :, :, 1:DH + 1],
                        mv_nat[:].rearrange("p mt (h d) -> p mt h d", d=DH))
                return g

            for tt in range(4):
                scan_groups.append(jac0(tt))
            for it in range(1, niter):
                for tt in range(4):
                    scan_groups.append(jac(it, tt))
            for ft in range(4):
                scan_groups.append(mk_group(ft))
            for mt in range(2):
                scan_groups.append(mv_group(mt))
            scan_groups.append(mv8_group())

            # ================= main pipeline stages =================
            def feat_chain(xt_parts, ft, feat16, feat8, act_light=False):
                ps = psA.tile([128, NB], f32, tag="psA")
                for j in range(16):
                    nc.tensor.matmul(
                        ps[:], wenc_t[j // 4][:, j % 4, ft * 128:(ft + 1) * 128],
                        xt_parts[j // 4][:, j % 4, :],
                        start=(j == 0), stop=(j == 15),
                    )
                f16t = ft_pool.tile([128, NB], f16, tag="ft16")
                if act_light:
                    nc.vector.tensor_copy(f16t[:], ps[:])
                else:
                    nc.scalar.copy(f16t[:], ps[:])
                feat16.append(f16t)
                nc.vector.tensor_copy(feat8[:, ft, :], ps[:])

            def qf_chain(feat8, ft, qf8):
                ps = psA.tile([128, NB], f32, tag="psA")
                for u2 in range(2):
                    nc.tensor.matmul(
                        ps[:], wq8_sb[:, 2 * u2:2 * u2 + 2,
                                      ft * 128:(ft + 1) * 128],
                        feat8[:, 2 * u2:2 * u2 + 2, :],
                        start=(u2 == 0), stop=(u2 == 1), perf_mode=DR,
                    )
                nc.vector.tensor_scalar_add(qf8[:, ft, :], ps[:],
                                            bq_sb[:, ft:ft + 1])

            def attn_setup(feat16):
                # logits1 accumulation + per-bt z column groups, one bank
                pz4 = psZ.tile([128, 4, 48], f32, tag="pz")
                pr4 = psV.tile([128, 4, H, 2], f16, tag="pr", bufs=1)
                for bt in range(4):
                    bsl = slice(bt * 128, (bt + 1) * 128)
                    for k in range(4):
                        nc.tensor.matmul(pz4[:, bt, 40:45],
                                         feat16[k][:, bsl], wclst_sb[:, k, :],
                                         start=(k == 0), stop=(k == 3))
                return dict(pz4=pz4, pr4=pr4, zq=[])

            def attn_scores(qf8, h, us):
                kf, p0 = h // 2, 64 * (h % 2)
                u8 = u8_pool.tile([128, 2, NB], f8, tag="u8")
                for mj in range(2):
                    ps = psS.tile([128, NB], f32, tag="psS")
                    nc.tensor.matmul(
                        ps[:],
                        mkt8[p0:p0 + 64, kf, mj * 128:(mj + 1) * 128],
                        qf8[p0:p0 + 64, kf, :], start=True, stop=True,
                    )
                    nc.scalar.activation(u8[:, mj, :], ps[:], FT.Exp,
                                         scale=INV_SQRT_DH)
                us.append((h, u8))

            def attn_value(h, u8, actx):
                pv = psV.tile([65, NB], f32, tag="psV")
                nc.tensor.matmul(pv[:], mv8aug[:, :, h, 0:DH + 1], u8[:],
                                 start=True, stop=True, perf_mode=DR)
                mo = mo_pool.tile([65, NB], f16, tag="mo")
                if h % 2 == 0 and not actx.get("light"):
                    nc.scalar.copy(mo[:], pv[:])
                else:
                    nc.vector.tensor_copy(mo[:], pv[:])
                actx["zq"].append((h, mo))

            def emit_z(h, mo, actx):
                pz4, pr4 = actx["pz4"], actx["pr4"]
                for bt in range(4):
                    bsl = slice(bt * 128, (bt + 1) * 128)
                    nc.tensor.transpose(pr4[:, bt, h, 0:1], mo[0:1, bsl],
                                        eye128_sb[0:1, 0:1])
                    nc.tensor.matmul(pz4[:, bt, 5 * h:5 * h + 5], mo[:, bsl],
                                     w2aug_sb[:, h, :], start=True, stop=True)

            def drain_z(actx, keep=0):
                zq = actx["zq"]
                while len(zq) > keep:
                    h, mo = zq.pop(0)
                    emit_z(h, mo, actx)

            def drain_values(us, actx, keep=0):
                while len(us) > keep:
                    h, u8 = us.pop(0)
                    attn_value(h, u8, actx)
                    drain_z(actx, keep=1)

            def attn_finish(actx, pch):
                # stage-major emission: consecutive DVE ops are independent
                # so they pipeline instead of paying the dep-chain latency
                pz4, pr4 = actx["pz4"], actx["pr4"]
                ysb = y_pool.tile([128, NCH, NW], f32, tag="ysb")
                rs, zss, tqs = [], [], []
                for bt in range(4):
                    r = r_pool.tile([128, 1, H], f32, tag="r")
                    nc.vector.reciprocal(
                        r[:],
                        pr4[:, bt, :, 0:1].rearrange("p h one -> p one h"))
                    rs.append(r)
                for bt in range(4):
                    zs = zs_pool.tile([128, NW, H], f32, tag="zs")
                    zv, rv = bass.broadcast_tensor_aps(
                        pz4[:, bt, 0:40].rearrange("p (h c) -> p c h", c=NW),
                        rs[bt][:])
                    nc.vector.tensor_mul(zs[:], zv, rv)
                    zss.append(zs)
                for bt in range(4):
                    tq = tq_pool.tile([128, NW], f32, tag="tq")
                    nc.vector.tensor_reduce(tq[:], zss[bt][:],
                                            axis=mybir.AxisListType.X,
                                            op=mybir.AluOpType.add)
                    tqs.append(tq)
                for bt in range(4):
                    nc.vector.tensor_add(ysb[:, bt, :], tqs[bt][:],
                                         pz4[:, bt, 40:45])
                nc.sync.dma_start(
                    y[pch * NB:(pch + 1) * NB, :].rearrange(
                        "(bt p) c -> p bt c", p=128),
                    ysb[:])

            # ================= software-pipelined macro loop =================
            # macro i: encoder+qf for chunk i; attention+z+y for chunk i-1
            # interleaved into the chains.  Macro 1 runs chunk 0's attention
            # in its qf phase (the feat phase carries the scan's serial
            # groups); the last encoder macro additionally runs its own
            # chunk's attention in its qf phase so the epilogue is only the
            # z-assembly.
            state = {}
            pending = list(scan_groups)

            for i in range(NMAC + 1):
                enc = i < NMAC
                last_enc = i == NMAC - 1
                ch = i % NCH
                pch = (i - 1) % NCH
                pstate = state
                state = {}
                us = []
                actx = None
                if i >= 2 and "us" not in pstate:
                    actx = attn_setup(pstate["feat16"])
                if enc:
                    xt_parts = xts[ch]
                    feat16 = []
                    feat8 = f8_pool.tile([128, 4, NB], f8, tag="ft8")
                    qf8 = qf_pool.tile([128, 4, NB], f8, tag="qf8")
                    state = dict(feat16=feat16, qf8=qf8)
                    for ft in range(4):
                        feat_chain(xt_parts, ft, feat16, feat8,
                                   act_light=(last_enc and O("light", 1) > 0))
                        if i == 1:
                            # interleave serial scan groups into the chains;
                            # attention must wait for them (mkt8/mv8aug deps)
                            for _ in range(5):
                                if pending:
                                    pending.pop(0)()
                        elif actx is not None:
                            drain_values(us, actx, keep=2)
                            attn_scores(pstate["qf8"], 2 * ft, us)
                            attn_scores(pstate["qf8"], 2 * ft + 1, us)
                    if i == 0:
                        scan_a()
                    if i == 1:
                        while pending:
                            pending.pop(0)()
                        actx = attn_setup(pstate["feat16"])
                    if actx is not None:
                        drain_values(us, actx, keep=2)
                    for ft in range(4):
                        qf_chain(feat8, ft, qf8)
                        if i == 1:
                            attn_scores(pstate["qf8"], 2 * ft, us)
                            attn_scores(pstate["qf8"], 2 * ft + 1, us)
                            drain_values(us, actx, keep=2)
                        elif actx is not None:
                            drain_values(us, actx, keep=2)
                    if actx is not None:
                        drain_values(us, actx, keep=0)
                        drain_z(actx, keep=0)
                        attn_finish(actx, pch)
                    if last_enc:
                        # run this (final) chunk's attention here so the
                        # epilogue macro is only the z-assembly
                        actx2 = attn_setup(feat16)
                        actx2["light"] = O("light", 1) > 0
                        us2 = []
                        for ft in range(4):
                            drain_values(us2, actx2, keep=2)
                            attn_scores(qf8, 2 * ft, us2)
                            attn_scores(qf8, 2 * ft + 1, us2)
                        state["us"] = us2
                        state["actx"] = actx2
                else:
                    # epilogue: finish the last chunk
                    drain_values(pstate["us"], pstate["actx"], keep=0)
                    drain_z(pstate["actx"], keep=0)
                    attn_finish(pstate["actx"], pch)

    nc.compile()
    return nc


def prep_inputs(inputs):
    """Host-side shard/layout prep. Returns per-core in_maps."""
    x = np.asarray(inputs["x"], dtype=np.float32)
    sx = np.asarray(inputs["support_x"], dtype=np.float32)
    sy = np.asarray(inputs["support_y"]).astype(np.int64)
    W_enc = np.asarray(inputs["W_enc"], dtype=np.float32)
    b_enc = np.asarray(inputs["b_enc"], dtype=np.float32)
    W_q = np.asarray(inputs["W_q"], dtype=np.float32)
    b_q = np.asarray(inputs["b_q"], dtype=np.float32)
    W_cls = np.asarray(inputs["W_cls"], dtype=np.float32)
    b_cls = np.asarray(inputs["b_cls"], dtype=np.float32)
    mem_keys = np.asarray(inputs["mem_keys"], dtype=np.float32)
    mem_values = np.asarray(inputs["mem_values"], dtype=np.float32)

    def pk(a, p=128):  # [K, N] -> [p, K/p, N] partition-major tiles
        k, n = a.shape
        return np.ascontiguousarray(a.reshape(k // p, p, n).transpose(1, 0, 2))

    # fold b_enc into the qf bias and the classifier bias (feat tiles are
    # produced without the encoder bias)
    bq_eff = b_enc @ W_q + b_q
    bcls_eff = b_cls + b_enc @ W_cls[:F]

    wenc_h = pk(_f16(W_enc))                     # [128, 16, F]
    sxt_h = pk(_f8(sx.T))                        # [128, 16, NS]
    wenc8_h = pk(_f8(W_enc))
    wq8_h = pk(_f8(W_q))                         # [128, 4, F]
    wclst_h = pk(_f16(W_cls[:F]))                # [128, 4, NW]
    w2aug_h = np.zeros((DH + 1, H, NW), np.float16)
    w2aug_h[0] = _f16(bcls_eff / H)[None, :]
    w2aug_h[1:] = _f16(W_cls[F:]).reshape(H, DH, NW).transpose(1, 0, 2)
    mkt = np.ascontiguousarray(mem_keys.T)       # [F, M]
    mkt0_h = pk(mkt)
    mkt0b_h = pk(_f8(mkt * INV_SQRT_F))
    mvals_h = pk(_f16(mem_values))               # [128, 2, F]
    vals = np.zeros((NS, NW), np.float32)
    vals[np.arange(NS), sy] = 1.0
    valsb_h = pk(_f16(vals))                     # [128, 4, NW]
    benc_h = np.ascontiguousarray(b_enc.reshape(4, 128).T)
    bq_h = np.ascontiguousarray(bq_eff.reshape(4, 128).T)

    shared = dict(
        wenc=wenc_h, wenc8=wenc8_h, sxt=sxt_h, wq8=wq8_h, wclst=wclst_h,
        w2aug=w2aug_h,
        mkt0=mkt0_h, mkt0b=mkt0b_h, mvals=mvals_h, valsb=valsb_h,
        benc=benc_h, bq=bq_h,
    )
    in_maps = []
    for c in range(NCORES):
        m = dict(shared)
        # x^T fp16, chunk-major tiles: [128p, NCH, 16j, NB]
        xc = _f16(x[c * BC:(c + 1) * BC].T)      # [D_IN, BC]
        m["xs"] = np.ascontiguousarray(
            xc.reshape(16, 128, NCH, NB).transpose(1, 2, 0, 3))
        in_maps.append(m)
    return in_maps


def kernel_ex(inputs, trace=False, **kwargs):
    nc = build()
    in_maps = prep_inputs(inputs)
    res = run_bass_kernel_spmd(nc, in_maps, core_ids=list(range(NCORES)),
                               trace=trace, **kwargs)
    out = np.concatenate([r["y"] for r in res.results], axis=0)
    return out.astype(np.float32), res


def kernel(**inputs):
    out, _ = kernel_ex(inputs)
    return out
